# revision 47
# baseline (speedup 1.0000x reference)
# Multi-head attention (B=2, S=2048, D=1024, H=16, head_dim=64) with bool mask,
# sharded across 8 TRN2 NeuronCores: core c -> batch c//4, heads 4*(c%4)..4*(c%4)+3.
#
# Per-core device kernel (scores computed transposed: scoresT[k, q]):
#   scoresT = K @ Q^T / 8           (PE, float32r, lhsT = K^T chunk, rhs = Q^T)
#   attnT   = exp(scoresT) * (1-m)T (ACT exp with scale=1/8 -> bf16, DVE mult)
#   outT'   = [V | 1]^T @ attnT     (PE, bf16; row 64 = softmax denominator Z)
#   out     = transpose(outT') / Z  (PE transpose + DVE per-partition scalar mult)
#
# Host side (inside kernel()): slice per-core shards, pre-transpose Q/K per head
# ([64, S] head-dim-major), pre-transpose the inverted mask to bf16, reassemble
# the 8 per-core outputs into the full [B, S, D] output.

import sys

import numpy as np

for _p in ("/opt/trn_rl_repo",):
    if _p not in sys.path:
        sys.path.insert(0, _p)

import ml_dtypes

import concourse.bass as bass
import concourse.tile as tile
from concourse import bacc, mybir
from concourse.bass_utils import run_bass_kernel_spmd
from concourse.masks import make_identity

F32 = mybir.dt.float32
F32R = mybir.dt.float32r
BF16 = mybir.dt.bfloat16

S = 2048          # sequence length
HD = 64           # head dim
HPC = 4           # heads per core
NCORES = 8
B = 2
H = 16
D = H * HD


def build_program(s=S, act_dtype=BF16, qk_dtype=BF16, n_psS=2):
    """Build the single-core SPMD program. Returns the compiled Bacc object."""
    nc = bacc.Bacc()

    KS = s // 128            # number of k strips
    QG = 1024 if s >= 1024 else s   # q group width (ACT/DVE instruction width)
    NQG = s // QG            # q groups
    NQC = max(QG // 512, 1)  # 512-wide matmul chunks per q group
    QC = min(512, QG)        # matmul chunk width
    JT = QG // 128           # out-transpose chunks per q group

    # q^T and k^T stacked so each head-pair loads with ONE DMA (a float32r
    # matmul can only carry one semaphore wait after walrus lowering).
    qkT_d = nc.declare_dram_parameter("qkT", [2, HPC * HD, s], qk_dtype, isOutput=False)
    v_d = nc.declare_dram_parameter("v", [s, HPC * HD], BF16, isOutput=False)
    nmT_d = nc.declare_dram_parameter("nmT", [s, s], BF16, isOutput=False)
    out_d = nc.declare_dram_parameter("out", [s, HPC * HD], BF16, isOutput=True)

    # DRAM views with the k/q axis split into strips of 128 partitions
    nm_view = nmT_d[:].rearrange("(ks p) q -> p ks q", p=128)
    v_view = v_d[:].rearrange("(ks p) c -> p ks c", p=128)
    out_view = out_d[:].rearrange("(sq p) c -> p sq c", p=128)

    with tile.TileContext(nc) as tc:
        with (
            tc.tile_pool(name="const", bufs=1) as const,
            tc.tile_pool(name="wq", bufs=1) as wq,
            tc.tile_pool(name="vstg", bufs=1) as vstg,
            tc.tile_pool(name="attn", bufs=20) as apool,
            tc.tile_pool(name="fin", bufs=2) as fpool,
            tc.tile_pool(name="stat", bufs=4) as spool,
            tc.tile_pool(name="oasm", bufs=1) as opool,
            tc.tile_pool(name="psS", bufs=n_psS, space="PSUM") as psS_pool,
            # psO (AV accumulator, [65,QG]=2 banks) and pn (out-transpose
            # target, [128,JT,128]=2 banks) share one tag with bufs=2: the
            # two slots alternate psO/pn roles, so AV of group g only waits
            # for the finalize reads of group g-2 (1.5 groups of slack).
            tc.tile_pool(name="psF", bufs=2, space="PSUM") as psF_pool,
        ):
            ident = const.tile([128, 128], F32)
            make_identity(nc, ident)

            # Q^T / K^T head pairs: [128, s] (head 2p on partitions 0-63,
            # head 2p+1 on partitions 64-127). Loads ride the scalar HWDGE
            # queue; mask strips ride the sync queue — two queues in parallel,
            # each emitted in the order compute consumes it. All DMA issues
            # precede the exp-table warmup on ACT (the ~1.3us table load
            # must not delay the load queue).
            qks = []
            for pair in range(HPC // 2):
                qk = wq.tile([128, 2, s], qk_dtype, tag=f"qkT{pair}")
                qks.append(qk)
            v_sb = vstg.tile([128, KS, HPC * HD], BF16)

            def qk_src(pair):
                return qkT_d[:, 128 * pair:128 * pair + 128, :].rearrange(
                    "t p s -> p t s"
                )

            # First pair's q and k halves ride different queues in parallel
            # (the first QK matmul needs both as early as possible). Mask
            # strips alternate queues so they land at ~2x the single-queue
            # rate — the first group consumes one strip per ~1us.
            nm_sb = wq.tile([128, KS, s], BF16, tag="nm")
            KH = KS // 2
            nc.scalar.dma_start(out=qks[0][:, 0, :], in_=qk_src(0)[:, 0, :])
            nc.sync.dma_start(out=qks[0][:, 1, :], in_=qk_src(0)[:, 1, :])
            nc.sync.dma_start(out=v_sb[:, :KH], in_=v_view[:, :KH])
            nc.sync.dma_start(out=v_sb[:, KH:], in_=v_view[:, KH:])
            for pair in range(1, HPC // 2):
                nc.scalar.dma_start(out=qks[pair], in_=qk_src(pair))
            for ks in range(KS):
                nc.sync.dma_start(out=nm_sb[:, ks, :], in_=nm_view[:, ks, :])

            # Preload the exp table while DMAs run.
            warm = const.tile([128, 1], F32)
            nc.vector.memset(warm, 0.0)
            nc.scalar.activation(warm, warm, mybir.ActivationFunctionType.Exp)

            # Warm the PE HAM clock gate while input DMAs run: ~3.5us of dummy
            # matmuls (transpose-mode doesn't count as PE-busy for HAM) so
            # the first real QKs run at 2.4GHz.
            zb = const.tile([128, 128], BF16)
            nc.vector.memset(zb, 0.0)
            for _ in range(24):
                wmm = psS_pool.tile([128, QG], F32, tag="psS")
                nc.tensor.matmul(
                    wmm[:, :128], lhsT=zb[0:64, :], rhs=zb[0:64, :],
                    start=True, stop=True,
                )

            # V' = [V | 1] per head, bf16; cast in halves so early AVs only
            # wait on the first half of the V DMA.
            vps = []
            for h in range(HPC):
                vp = wq.tile([128, KS, HD + 1], BF16, tag=f"vp{h}")
                vps.append(vp)
            for half in range(2):
                ksl = slice(half * KH, KH + half * KH)
                for h in range(HPC):
                    nc.vector.tensor_copy(
                        out=vps[h][:, ksl, 0:HD],
                        in_=v_sb[:, ksl, h * HD:(h + 1) * HD],
                    )
                    nc.vector.memset(vps[h][:, ksl, HD:HD + 1], 1.0)

            out_asm = opool.tile([128, KS, HPC * HD], BF16)

            # Emission state threading three overlapped group pipelines:
            #   carry — group awaiting its last AV (stop=True) + psO->oT copy
            #   pend  — group awaiting its JT transpose+normalize steps
            fin = {"pend": None, "idx": 0, "pn": None}
            N_FIN = JT + 1  # JT transposes + one batched normalize step

            def finalize_step():
                """Emit one finalize chunk of a finished q-group: steps
                0..JT-1 transpose [65,128] pieces into pn; step JT does one
                strided reciprocal over the JT Z values and two broadcast
                multiplies (batched — avoids per-strip sequencer overhead)."""
                h, qg, oT = fin["pend"]
                if fin["idx"] >= N_FIN:
                    return
                j = fin["idx"]
                fin["idx"] += 1
                if j == 0:
                    pn_t = psF_pool.tile([128, JT, 128], F32, tag="fin")
                    fin["pn"] = pn_t
                pn = fin["pn"]
                if j < JT:
                    nc.tensor.transpose(
                        pn[:, j, :HD + 1],
                        oT[:, j * 128:(j + 1) * 128],
                        ident[:HD + 1, :HD + 1],
                    )
                    return
                rec8 = spool.tile([128, JT], F32)
                nc.vector.reciprocal(rec8, pn[:, :, HD])
                half = (JT + 1) // 2
                for lo in range(0, JT, half):
                    hi = min(lo + half, JT)
                    sq0 = qg * JT + lo
                    nc.vector.tensor_mul(
                        out_asm[:, sq0:sq0 + hi - lo, h * HD:(h + 1) * HD],
                        pn[:, lo:hi, 0:HD],
                        rec8[:, lo:hi].to_broadcast([128, hi - lo, HD]),
                    )
                    if h == HPC - 1:
                        eng = nc.sync if lo == 0 else nc.scalar
                        eng.dma_start(
                            out=out_view[:, sq0:sq0 + hi - lo, :],
                            in_=out_asm[:, sq0:sq0 + hi - lo, :],
                        )

            def emit_carry(carry):
                """Last AV (stop=True) + psO->SBUF copy for a finished group."""
                ch, cqg, cpsO, cat = carry
                for qc in range(NQC):
                    nc.tensor.matmul(
                        cpsO[:, qc * QC:(qc + 1) * QC],
                        lhsT=vps[ch][:, KS - 1, :],
                        rhs=cat[:, qc * QC:(qc + 1) * QC],
                        start=(KS == 1),
                        stop=True,
                    )
                oT = fpool.tile([HD + 1, QG], F32, tag="oT")
                nc.vector.tensor_copy(oT, cpsO)
                # flush any unfinished finalize steps of the older group
                while fin["pend"] is not None and fin["idx"] < N_FIN:
                    finalize_step()
                fin["pend"] = (ch, cqg, oT)
                fin["idx"] = 0

            carry = None
            groups = [(h, qg) for h in range(HPC) for qg in range(NQG)]
            for h, qg in groups:
                base = 64 * (h % 2)
                qt_r = qks[h // 2][:, 0, :]
                kt_r = qks[h // 2][:, 1, :]
                q0 = qg * QG
                psO = None
                at_prev = None
                for ks in range(KS):
                    # AV one strip behind QK, emitted BEFORE this strip's QK
                    # so it isn't queued behind QK's psum-slot wait on PE.
                    if at_prev is not None:
                        if psO is None:
                            psO = psF_pool.tile([HD + 1, QG], F32, tag="fin")
                        for qc in range(NQC):
                            nc.tensor.matmul(
                                psO[:, qc * QC:(qc + 1) * QC],
                                lhsT=vps[h][:, ks - 1, :],
                                rhs=at_prev[:, qc * QC:(qc + 1) * QC],
                                start=(ks == 1),
                                stop=False,
                            )
                    # Transpose+normalize of an older group, interleaved so
                    # it never stalls the PE pipeline.
                    if fin["pend"] is not None and ks >= 1:
                        finalize_step()
                    psS = psS_pool.tile([128, QG], F32)
                    for qc in range(NQC):
                        nc.tensor.matmul(
                            psS[:, qc * QC:(qc + 1) * QC],
                            lhsT=kt_r[base:base + HD, ks * 128:(ks + 1) * 128],
                            rhs=qt_r[base:base + HD, q0 + qc * QC:q0 + (qc + 1) * QC],
                            start=True,
                            stop=True,
                        )
                    if ks == 0 and carry is not None:
                        emit_carry(carry)
                        carry = None
                    at = apool.tile([128, QG], act_dtype, tag="at")
                    nc.scalar.activation(
                        at, psS, mybir.ActivationFunctionType.Exp, scale=0.125
                    )
                    nc.vector.tensor_mul(at, at, nm_sb[:, ks, q0:q0 + QG])
                    at_prev = at
                carry = (h, qg, psO, at_prev)
            emit_carry(carry)
            while fin["idx"] < N_FIN:
                finalize_step()
    nc.compile()
    return nc


_CACHE = {}


def _get_nc():
    if "nc" not in _CACHE:
        _CACHE["nc"] = build_program()
    return _CACHE["nc"]


def make_in_maps(q, k, v, mask, s=S):
    """Shard full inputs into 8 per-core input maps (host-side layout prep)."""
    q = np.asarray(q, dtype=np.float32)
    k = np.asarray(k, dtype=np.float32)
    v = np.asarray(v, dtype=np.float32)
    mask = np.asarray(mask)
    nh = q.shape[-1] // HD
    in_maps = []
    for c in range(NCORES):
        b, g = divmod(c, NCORES // B)
        h0 = HPC * g
        qs = q[b].reshape(s, nh, HD)[:, h0:h0 + HPC, :]      # [s, HPC, 64]
        ks_ = k[b].reshape(s, nh, HD)[:, h0:h0 + HPC, :]
        qkT = np.empty((2, HPC * HD, s), ml_dtypes.bfloat16)
        qkT[0] = qs.transpose(1, 2, 0).reshape(HPC * HD, s)
        qkT[1] = ks_.transpose(1, 2, 0).reshape(HPC * HD, s)
        vc = np.ascontiguousarray(v[b, :, h0 * HD:(h0 + HPC) * HD]).astype(
            ml_dtypes.bfloat16
        )
        nmT = np.ascontiguousarray((~mask[b]).T).astype(ml_dtypes.bfloat16)
        in_maps.append({"qkT": qkT, "v": vc, "nmT": nmT})
    return in_maps


def assemble_out(results, s=S, d=D):
    out = np.empty((B, s, d), np.float32)
    for c in range(NCORES):
        b, g = divmod(c, NCORES // B)
        out[b, :, g * HPC * HD:(g + 1) * HPC * HD] = results[c]["out"]
    return out


def kernel(q, k, v, mask):
    nc = _get_nc()
    in_maps = make_in_maps(q, k, v, mask)
    res = run_bass_kernel_spmd(nc, in_maps, list(range(NCORES))).results
    return assemble_out(res)
